# revision 17
# baseline (speedup 1.0000x reference)
"""Cost-sensitive cross-entropy loss on 8 Trainium2 NeuronCores.

Strategy (data-parallel over batch):
  - Each of the 8 cores processes a 16384-row shard of `outputs` [131072, 1000].
  - Per 128-row tile the device computes: row-wise sum(exp(x)) (ScalarE Exp with
    accumulate) and row-wise argmax (VectorE max -> max_index, first-occurrence
    semantics matching jnp.argmax).  No max-subtraction is needed: inputs are
    O(1)-scaled so exp() cannot overflow fp32, and skipping it decouples ACT
    from DVE for full engine overlap.
  - Host combines: lse = log(sumexp), glp = x[i,t_i] - lse (gather), scatter-add
    count matrix from (target, predicted), cost-matrix transform, final scalar.
    All host math is O(B + C^2) ~ 1e6 elements vs the 5e8-element device pass.
"""

import contextlib
import sys

import numpy as np

try:
    import concourse.bass as bass
except ImportError:  # stand-alone grading dir: fall back to the repo install
    for p in ("/opt/trn_rl_repo", "/root/.axon_site/_ro/trn_rl_repo"):
        if p not in sys.path:
            sys.path.insert(0, p)
    import concourse.bass as bass

import concourse.mybir as mybir
from concourse import bass_utils

B, C = 131072, 1000
NCORES = 8
SHARD = B // NCORES  # 16384 rows per core
P = 128              # SBUF partitions = rows per tile
NT = SHARD // P      # 128 tiles per core
BETA1, BETA2 = 1.0, 2.0

_nc_cache = {}


def build_bass(nt: int = NT, sx: int = 8, se: int = 8):
    """One-core program; SPMD-replicated across all 8 cores.

    Raw Bass (no Tile): this walrus build allows at most one embedded sync
    wait per instruction, so all waits are standalone wait_ge instructions
    on the issuing engine's stream.

    Pipeline per 128-row tile t:
      SP:  [wait exp(t-sx) done]  dma xt[t%sx] <- x rows  (+16 dma_sem[slot])
      ACT: [wait load t landed; wait argmax(t-se) done]
           ex[t%se] = exp(xt), accum -> stage_s[:, t]     (+1 act_sem)
      DVE: [wait exp t done] max8 -> max_index -> stage_i (+1 dve_sem)
    """
    key = (nt, sx, se)
    if key in _nc_cache:
        return _nc_cache[key]

    f32 = mybir.dt.float32
    u32 = mybir.dt.uint32
    Exp = mybir.ActivationFunctionType.Exp

    nc = bass.Bass()
    x = nc.declare_dram_parameter("x", [nt * P, C], f32, isOutput=False)
    s_out = nc.declare_dram_parameter("s_out", [P, nt], f32, isOutput=True)
    i_out = nc.declare_dram_parameter("i_out", [P, nt * 8], u32, isOutput=True)

    with contextlib.ExitStack() as ctx:
        xt = [ctx.enter_context(nc.sbuf_tensor(f"xt{i}", [P, C], f32))
              for i in range(sx)]
        ex = [ctx.enter_context(nc.sbuf_tensor(f"ex{i}", [P, C], f32))
              for i in range(se)]
        mx8 = [ctx.enter_context(nc.sbuf_tensor(f"mx8_{i}", [P, 8], f32))
               for i in range(4)]
        stage_s = ctx.enter_context(nc.sbuf_tensor("stage_s", [P, nt], f32))
        stage_i = ctx.enter_context(
            nc.sbuf_tensor("stage_i", [P, nt * 8], u32))
        dma_sem = [ctx.enter_context(nc.semaphore(f"dma_sem{i}"))
                   for i in range(sx)]
        out_sem = ctx.enter_context(nc.semaphore("out_sem"))
        act_sem = ctx.enter_context(nc.semaphore("act_sem"))
        dve_sem = ctx.enter_context(nc.semaphore("dve_sem"))
        vsem = ctx.enter_context(nc.semaphore("vsem"))
        block = ctx.enter_context(nc.Block())

        @block.sync
        def _(sync):
            for t in range(nt):
                if t >= sx:
                    # exp(t-sx) read slot t%sx -> earlier load fully consumed
                    sync.wait_ge(act_sem, t - sx + 1)
                sync.dma_start(
                    out=xt[t % sx][:], in_=x[t * P:(t + 1) * P, :]
                ).then_inc(dma_sem[t % sx], 16)
            sync.wait_ge(act_sem, nt)
            sync.dma_start(out=s_out[:, :], in_=stage_s[:]).then_inc(out_sem, 16)
            sync.wait_ge(dve_sem, nt)
            sync.dma_start(out=i_out[:, :], in_=stage_i[:]).then_inc(out_sem, 16)
            sync.wait_ge(out_sem, 32)

        @block.scalar
        def _(scalar):
            for t in range(nt):
                scalar.wait_ge(dma_sem[t % sx], 16 * (t // sx + 1))
                if t >= se:
                    # argmax(t-se) read ex slot t%se -> slot free
                    scalar.wait_ge(dve_sem, t - se + 1)
                scalar.activation(
                    ex[t % se][:], xt[t % sx][:], Exp,
                    accum_out=stage_s[:, t:t + 1],
                ).then_inc(act_sem, 1)

        @block.vector
        def _(vector):
            # 2-stage SW pipeline: max(t) runs 2 tiles ahead of
            # max_index(t-2), so every sem wait (RAW on mx8, WAR on mx8
            # slot recycle) is satisfied well before it executes -- no DVE
            # pipeline drain.
            def mi(t):
                vector.wait_ge(vsem, t + 1)
                vector.max_index(
                    stage_i[:, t * 8:(t + 1) * 8], mx8[t % 4][:],
                    ex[t % se][:],
                ).then_inc(dve_sem, 1)

            for t in range(nt):
                if t >= 4:
                    vector.wait_ge(dve_sem, t - 3)  # mi(t-4) freed mx8 slot
                vector.wait_ge(act_sem, t + 1)
                vector.max(mx8[t % 4][:], ex[t % se][:]).then_inc(vsem, 1)
                if t >= 2:
                    mi(t - 2)
            mi(nt - 2)
            mi(nt - 1)

    _nc_cache[key] = nc
    return nc


def run_device(outputs: np.ndarray, trace: bool = False, **kw):
    """Run the SPMD kernel; returns (sumexp [B], pred [B], BassKernelResults)."""
    nc = build_bass()
    xs = outputs.reshape(NCORES, SHARD, C)
    in_maps = [{"x": np.ascontiguousarray(xs[i])} for i in range(NCORES)]
    br = bass_utils.run_bass_kernel_spmd(
        nc, in_maps, list(range(NCORES)), trace=trace, **kw
    )
    res = br.results
    sumexp = np.empty((NCORES, SHARD), np.float64)
    pred = np.empty((NCORES, SHARD), np.int64)
    for i in range(NCORES):
        # staging layout is [partition p, tile t]; flat row index = t*P + p
        sumexp[i] = res[i]["s_out"].astype(np.float64).T.reshape(-1)
        idx0 = res[i]["i_out"].reshape(P, NT, 8)[:, :, 0]
        pred[i] = idx0.T.reshape(-1).astype(np.int64)
    return sumexp.reshape(-1), pred.reshape(-1), br


def finish_host(outputs, targets, cost_matrix, sumexp, pred):
    t = np.asarray(targets).astype(np.int64)
    lse = np.log(sumexp)
    tlogit = outputs[np.arange(B), t].astype(np.float64)
    glp = tlogit - lse

    counts = np.bincount(t * C + pred, minlength=C * C).reshape(C, C)
    cm = cost_matrix.astype(np.float64) + counts
    cm = cm ** 0.25
    np.fill_diagonal(cm, 0.0)
    cm = cm * (BETA2 / cm.max())
    cm = np.clip(cm, BETA1, BETA2)
    gc = cm[t, pred]

    loss = -(glp.mean() * gc.mean())
    return np.asarray(loss, dtype=np.float32)


def kernel(outputs, targets, cost_matrix):
    outputs = np.asarray(outputs)
    sumexp, pred, _ = run_device(outputs)
    return finish_host(outputs, np.asarray(targets), np.asarray(cost_matrix),
                       sumexp, pred)
